# revision 33
# baseline (speedup 1.0000x reference)
"""Channel-attention block (AttentionBlock, C=64) on 8 trn2 NeuronCores.

Algebraic reduction: with q = wq x + bq etc. and attention over channels,
    S  = q k^T / sqrt(C) = wqa^T_aug G_aug wka_aug / 8,   G_aug = [[x x^T, s],[s^T, N]]
    out = softmax(S) v + x = (attn wv + I) x + (attn bv) 1^T
so the kernel only needs the 65x65 Gram (per batch) of x plus one matmul pass
over x.  The N axis is sharded over 8 cores; the [G|s] partial sums (33 KB)
are AllReduce'd on-device.

Layout: batches stacked on partitions (p = b*64 + c) so matmuls run K=M=128
with block-diagonal weights.  The Gram contraction needs n on partitions, so
the host supplies an fp16 copy of x pre-permuted to [p, q, c] (q indexes
128-position chunks) with a ones channel appended, so the Gram + row-sums
accumulate in one fp16 matmul per chunk with zero on-device transposes.
Phase 2 also runs on a natural fp16 x (the +x residual flows through the
identity inside Q), so no fp32 x is ever moved.  Output is stored fp16 and
upcast on host, so total DMA is 8.4 MB xh + 8.4 MB x + 8.4 MB out per core.

Changes vs the 148-157us baseline:
  - out stored fp16 (halves the phase-2 store traffic; host upcasts)
  - input DMAs ride the sync queue, stores ride gpsimd
  - AllReduce (Shared-output) replaces AllGather + on-device rank-reduce
  - S = wqa^T G_aug wka is linear in G_aug, so each core computes its
    S-partial locally BEFORE the collective and the AllReduce carries S
    [128, 64] directly; the post-collective path is just softmax + QT
    assembly.  The collective starts ~11.6us after the LAST core's
    trigger, so the pre-trigger path is compressed: growing slab
    schedule starts the (PE-paced) gram earlier, and the S-partial
    copy chain is split across the vector and scalar engines with the
    collective input stored in per-batch halves.
"""

import ml_dtypes
import numpy as np

import concourse.bacc as bacc
import concourse.mybir as mybir
import concourse.tile as tile
from concourse import bass_utils

F32 = mybir.dt.float32
F32R = mybir.dt.float32r
BF16 = mybir.dt.bfloat16
F16 = mybir.dt.float16

NCORES = 8
B, C = 2, 64
P = B * C  # 128 partitions, batches stacked
N_TOTAL = 64 * 64 * 64  # 262144
N_SHARD = N_TOTAL // NCORES  # 32768
GCHUNK = 128
N_GCH = N_SHARD // GCHUNK  # 256
# growing slab schedule: the gram is PE-paced at ~114ns/chunk from its first
# matmul, so a small first slab starts the PE ~2.5us earlier; later slabs grow
# so DMA completion (~107ns/chunk) stays ahead of PE consumption
SLABS = [8, 8, 8, 12, 12, 16, 16, 20, 20, 24, 24, 28, 28, 32]
assert sum(SLABS) == N_GCH
OCHUNK = 512  # phase-2 matmul free dim
OSTORE = 1024  # output store width (2 KB/partition line in fp16)
LDCHUNK = 2048  # fp16 input DMA slice
N_LDCH = N_SHARD // LDCHUNK  # 16


def build_bass():
    nc = bacc.Bacc(
        "TRN2",
        target_bir_lowering=False,
        debug=False,
        num_devices=NCORES,
    )

    x_t = nc.dram_tensor("x", [P, N_SHARD], F16, kind="ExternalInput")
    xh_t = nc.dram_tensor("xh", [P, N_GCH, GCHUNK + 1], F16, kind="ExternalInput")
    wqa_t = nc.dram_tensor("wqa", [65, 64], F32, kind="ExternalInput")  # [wq|bq]^T/8
    wka_t = nc.dram_tensor("wka", [65, 64], F32, kind="ExternalInput")  # [wk|bk]^T
    wv_t = nc.dram_tensor("wv", [64, 64], F32, kind="ExternalInput")
    bv_t = nc.dram_tensor("bv", [64, 1], F32, kind="ExternalInput")
    id_t = nc.dram_tensor("ident", [128, 128], F32, kind="ExternalInput")
    out_t = nc.dram_tensor("out", [P, N_SHARD], F16, kind="ExternalOutput")

    with tile.TileContext(nc, num_cores=NCORES) as tc:
        with (
            tc.tile_pool(name="xbuf", bufs=1) as xpool,
            tc.tile_pool(name="consts", bufs=1) as cpool,
            tc.tile_pool(name="slab", bufs=6) as spool,
            tc.tile_pool(name="osb", bufs=6) as opool,
            tc.tile_pool(name="dram", bufs=2, space="DRAM") as dram,
        ):
            # ---- first gram slab before anything else ----
            slab_tiles = []
            slab0 = spool.tile([P, SLABS[0], GCHUNK + 1], F16, tag="slab0", bufs=1)
            nc.sync.dma_start(slab0[:], xh_t[:, 0 : SLABS[0], :])
            slab_tiles.append(slab0)
            xs = xpool.tile([P, N_SHARD], F16)

            # ---- constants to SBUF ----
            ident = cpool.tile([128, 128], F32)
            nc.scalar.dma_start(ident[:], id_t[:, :])
            wqa = cpool.tile([65, 64], F32)
            nc.scalar.dma_start(wqa[:], wqa_t[:, :])
            wka = cpool.tile([65, 64], F32)
            nc.scalar.dma_start(wka[:], wka_t[:, :])
            wv = cpool.tile([64, 64], F32)
            nc.scalar.dma_start(wv[:], wv_t[:, :])
            bv = cpool.tile([64, 1], F32)
            nc.scalar.dma_start(bv[:], bv_t[:, :])

            zeros_f = cpool.tile([128, 128], F32)
            nc.vector.memset(zeros_f[:], 0.0)
            qt_r = cpool.tile([128, 128], F16)

            # ---- phase 1: G_psum[:,0:128] += xT^T xT ; col 128 = row sums ----
            gs = cpool.tile([P, 65], F32)
            with tc.tile_pool(name="gacc", bufs=1, space="PSUM") as gpool:
                # host appends a ones channel to xh, so one accumulation chain
                # yields [G | s] together
                g_ps = gpool.tile([P, GCHUNK + 1], F32)
                base = SLABS[0]
                for t, L in enumerate(SLABS[1:], start=1):
                    slab = spool.tile(
                        [P, L, GCHUNK + 1], F16, tag=f"slab{t}", bufs=1,
                        name=f"slab_{t}",
                    )
                    nc.sync.dma_start(slab[:], xh_t[:, base : base + L, :])
                    slab_tiles.append(slab)
                    base += L
                for k in range(N_LDCH):
                    sl = slice(k * LDCHUNK, (k + 1) * LDCHUNK)
                    nc.sync.dma_start(xs[:, sl], x_t[:, sl])
                j = 0
                for slab, L in zip(slab_tiles, SLABS):
                    for q in range(L):
                        nc.tensor.matmul(
                            g_ps[:],
                            lhsT=slab[:, q, 0:GCHUNK],
                            rhs=slab[:, q, :],
                            start=(j == 0),
                            stop=(j == N_GCH - 1),
                        )
                        j += 1
                # split the small copies across vector and scalar so the
                # serial chain to the collective trigger is ~2us shorter
                nc.vector.tensor_copy(gs[:, 64:65], g_ps[:, 128:129])
                nc.vector.tensor_copy(gs[0:64, 0:64], g_ps[0:64, 0:64])
                nc.scalar.add(gs[64:128, 0:64], g_ps[64:128, 64:128], 0.0)

            # ---- local S partial: S_p = wqa^T G_aug_p wka (linear in G) ----
            mpool = tc.alloc_tile_pool(name="pmath", bufs=1, space="PSUM")
            # s^T row via PE transpose of the s column
            st_ps = mpool.tile([1, 128], F32, tag="m1")
            nc.tensor.transpose(st_ps[:], gs[:, 64:65], ident[:])
            st = cpool.tile([1, 128], F32)
            nc.vector.tensor_copy(st[:], st_ps[:])

            ga = []
            for b in range(B):
                g_aug = cpool.tile([65, 65], F32, tag=f"ga{b}", name=f"g_aug{b}")
                cs = slice(b * 64, (b + 1) * 64)
                nc.vector.tensor_copy(g_aug[0:64, 0:64], gs[cs, 0:64])
                nc.vector.tensor_copy(g_aug[0:64, 64:65], gs[cs, 64:65])
                nc.vector.tensor_copy(g_aug[64:65, 0:64], st[:, cs])
                nc.vector.memset(g_aug[64:65, 64:65], float(N_SHARD))
                ga.append(g_aug)

            # A_b = G_aug_b @ wka  (G_aug symmetric -> lhsT = G_aug)
            cc_in = dram.tile([P, 64], F32)
            cc_out = dram.tile([P, 64], F32, addr_space="Shared")
            s_ps = mpool.tile([P, 64], F32, tag="m2")
            s_sb = cpool.tile([P, 64], F32)
            for b in range(B):
                cs = slice(b * 64, (b + 1) * 64)
                a_ps = mpool.tile([65, 64], F32, tag="m1", name=f"a_ps{b}")
                nc.tensor.matmul(a_ps[:], lhsT=ga[b][:], rhs=wka[:])
                a_sb = cpool.tile([65, 64], F32, tag=f"asb{b}", name=f"a_sb{b}")
                if b == 0:
                    nc.vector.tensor_copy(a_sb[:], a_ps[:])
                else:
                    nc.scalar.add(a_sb[:], a_ps[:], 0.0)
                # S_b = wqa^T @ A_b   (1/8 scale folded into wqa)
                nc.tensor.matmul(s_ps[cs, :], lhsT=wqa[:], rhs=a_sb[:])
                if b == 0:
                    nc.vector.tensor_copy(s_sb[cs, :], s_ps[cs, :])
                else:
                    nc.scalar.add(s_sb[cs, :], s_ps[cs, :], 0.0)
                # half-store so the first 16KB rides the queue while the
                # second half's copy finishes
                nc.scalar.dma_start(cc_in[cs, :], s_sb[cs, :])

            # ---- AllReduce the S partials (summed in-network) ----
            nc.gpsimd.collective_compute(
                "AllReduce",
                mybir.AluOpType.add,
                replica_groups=[list(range(NCORES))],
                ins=[cc_in.opt()],
                outs=[cc_out.opt()],
            )
            sr = cpool.tile([P, 64], F32)
            # halves on two queues: both DGE latencies run concurrently
            nc.scalar.dma_start(sr[0:64, :], cc_out[0:64, :])
            nc.sync.dma_start(sr[64:128, :], cc_out[64:128, :])

            # softmax rows (both batches stacked [128, 64])
            negmax = cpool.tile([P, 1], F32)
            nc.vector.reduce_max(
                negmax[:], sr[:], axis=mybir.AxisListType.X, negate=True
            )
            expv = cpool.tile([P, 64], F32)
            rowsum = cpool.tile([P, 1], F32)
            nc.scalar.activation(
                expv[:], sr[:], mybir.ActivationFunctionType.Exp,
                bias=negmax[:, 0:1], scale=1.0, accum_out=rowsum[:, 0:1],
            )
            rinv = cpool.tile([P, 1], F32)
            nc.vector.reciprocal(rinv[:], rowsum[:])
            attn = cpool.tile([P, 64], F32)
            nc.vector.tensor_scalar_mul(attn[:], expv[:], rinv[:, 0:1])

            # attn^T (one transpose: [128,64] -> [64,128] = [attn0^T | attn1^T])
            at_ps = mpool.tile([64, 128], F32, tag="m1")
            nc.tensor.transpose(at_ps[:], attn[:], ident[:])
            at_sb = cpool.tile([64, 128], F32)
            nc.vector.tensor_copy(at_sb[:], at_ps[:])

            # QT block-diag [128,128]: QT_b = wv^T attn_b^T + I
            qt_ps = mpool.tile([128, 128], F32, tag="m2")
            c_ps = mpool.tile([128, 1], F32, tag="m3")
            for b in range(B):
                cs = slice(b * 64, (b + 1) * 64)
                nc.tensor.matmul(
                    qt_ps[cs, cs], lhsT=wv[:], rhs=at_sb[:, cs],
                    start=True, stop=False,
                )
                nc.tensor.matmul(
                    qt_ps[cs, cs], lhsT=ident[0:64, 0:64], rhs=ident[0:64, 0:64],
                    start=False, stop=True,
                )
                nc.tensor.matmul(c_ps[cs, :], lhsT=at_sb[:, cs], rhs=bv[:])
            nc.vector.tensor_copy(qt_r[0:64, 64:128], zeros_f[0:64, 64:128])
            nc.vector.tensor_copy(qt_r[64:128, 0:64], zeros_f[64:128, 0:64])
            for b in range(B):
                cs = slice(b * 64, (b + 1) * 64)
                nc.vector.tensor_copy(qt_r[cs, cs], qt_ps[cs, cs])
            cvec = cpool.tile([P, 1], F32)
            nc.vector.tensor_copy(cvec[:], c_ps[:])
            mpool.release()

            # ---- phase 2: out = QT^T x + c  (fp16 matmuls, fp16 stores) ----
            # copies split 30/34 vector/scalar (scalar's per-chunk copy is
            # slightly cheaper, so it takes the extra chunks); every store's
            # two chunks still land on different engines
            with tc.tile_pool(name="ops", bufs=6, space="PSUM") as oppool:
                ci = 0
                for k in range(N_SHARD // OSTORE):
                    osb = opool.tile([P, OSTORE], F16, tag="osb", name="osb")
                    for h in range(OSTORE // OCHUNK):
                        sl = slice(k * OSTORE + h * OCHUNK, k * OSTORE + (h + 1) * OCHUNK)
                        o_ps = oppool.tile([P, OCHUNK], F32, tag="o", name="o_ps")
                        nc.tensor.matmul(o_ps[:], lhsT=qt_r[:], rhs=xs[:, sl])
                        oslice = osb[:, h * OCHUNK : (h + 1) * OCHUNK]
                        # every 16th pair goes scalar+scalar -> 30/34 split
                        on_vector = (h % 2 == 0) and (ci % 32 != 0)
                        if on_vector:
                            nc.vector.tensor_scalar_add(oslice, o_ps[:], cvec[:, 0:1])
                        else:
                            nc.scalar.add(oslice, o_ps[:], cvec[:, 0:1])
                        ci += 1
                    nc.gpsimd.dma_start(
                        out_t[:, k * OSTORE : (k + 1) * OSTORE], osb[:]
                    )

    nc.compile()
    return nc


_cached_nc = None


def kernel(x, wq, bq, wk, bk, wv, bv, _trace=False):
    global _cached_nc
    x = np.ascontiguousarray(np.asarray(x, dtype=np.float32))
    assert x.shape == (B, C, 64, 64, 64)
    xf = x.reshape(P, N_TOTAL)

    wqa = (
        np.concatenate(
            [np.asarray(wq, np.float64), np.asarray(bq, np.float64)[:, None]], axis=1
        ).T
        / 8.0
    ).astype(np.float32)  # [65, 64]
    wka = (
        np.concatenate(
            [np.asarray(wk, np.float64), np.asarray(bk, np.float64)[:, None]], axis=1
        ).T
    ).astype(np.float32)  # [65, 64]
    wv32 = np.ascontiguousarray(np.asarray(wv, np.float32))
    bv32 = np.ascontiguousarray(np.asarray(bv, np.float32).reshape(64, 1))
    ident = np.eye(128, dtype=np.float32)

    in_maps = []
    for i in range(NCORES):
        sl = slice(i * N_SHARD, (i + 1) * N_SHARD)
        xsh = np.ascontiguousarray(xf[:, sl].astype(np.float16))
        # xh[p, q, c] = x[c, q*128 + p] in fp16 (gram operand, n on partitions)
        xh = xsh.astype(np.float16).reshape(P, N_GCH, GCHUNK).transpose(2, 1, 0)
        xh = np.ascontiguousarray(
            np.concatenate(
                [xh, np.ones((GCHUNK, N_GCH, 1), np.float16)], axis=2
            )
        )
        in_maps.append(
            {
                "x": xsh,
                "xh": xh,
                "wqa": wqa,
                "wka": wka,
                "wv": wv32,
                "bv": bv32,
                "ident": ident,
            }
        )

    if _cached_nc is None:
        _cached_nc = build_bass()
    nc = _cached_nc

    res = bass_utils.run_bass_kernel_spmd(
        nc, in_maps, core_ids=list(range(NCORES)), trace=_trace
    )
    kernel._last_results = res

    out = np.empty((P, N_TOTAL), dtype=np.float32)
    for i in range(NCORES):
        out[:, i * N_SHARD : (i + 1) * N_SHARD] = res.results[i]["out"].astype(
            np.float32
        )
    return out.reshape(B, C, 64, 64, 64)


kernel._last_results = None


# revision 34
# speedup vs baseline: 1.4905x; 1.4905x over previous
"""Channel-attention block (AttentionBlock, C=64) on 8 trn2 NeuronCores.

Algebraic reduction: with q = wq x + bq etc. and attention over channels,
    S  = q k^T / sqrt(C) = wqa^T_aug G_aug wka_aug / 8,   G_aug = [[x x^T, s],[s^T, N]]
    out = softmax(S) v + x = (attn wv + I) x + (attn bv) 1^T
so the kernel only needs the 65x65 Gram (per batch) of x plus one matmul pass
over x.  The N axis is sharded over 8 cores; the [G|s] partial sums (33 KB)
are AllReduce'd on-device.

Layout: batches stacked on partitions (p = b*64 + c) so matmuls run K=M=128
with block-diagonal weights.  The Gram contraction needs n on partitions, so
the host supplies an fp16 copy of x pre-permuted to [p, q, c] (q indexes
128-position chunks) with a ones channel appended, so the Gram + row-sums
accumulate in one fp16 matmul per chunk with zero on-device transposes.
Phase 2 also runs on a natural fp16 x (the +x residual flows through the
identity inside Q), so no fp32 x is ever moved.  Output is stored fp16 and
upcast on host, so total DMA is 8.4 MB xh + 8.4 MB x + 8.4 MB out per core.

Changes vs the 148-157us baseline:
  - out stored fp16 (halves the phase-2 store traffic; host upcasts)
  - input DMAs ride the sync queue, stores ride gpsimd
  - AllReduce (Shared-output) replaces AllGather + on-device rank-reduce
  - S = wqa^T G_aug wka is linear in G_aug, so each core computes its
    S-partial locally BEFORE the collective and the AllReduce carries S
    [128, 64] directly; the post-collective path is just softmax + QT
    assembly.  The collective starts ~11.6us after the LAST core's
    trigger, so the pre-trigger path is compressed: growing slab
    schedule starts the (PE-paced) gram earlier, and the S-partial
    copy chain is split across the vector and scalar engines with the
    collective input stored in per-batch halves.
"""

import ml_dtypes
import numpy as np

import concourse.bacc as bacc
import concourse.mybir as mybir
import concourse.tile as tile
from concourse import bass_utils

F32 = mybir.dt.float32
F32R = mybir.dt.float32r
BF16 = mybir.dt.bfloat16
F16 = mybir.dt.float16

NCORES = 8
B, C = 2, 64
P = B * C  # 128 partitions, batches stacked
N_TOTAL = 64 * 64 * 64  # 262144
N_SHARD = N_TOTAL // NCORES  # 32768
GCHUNK = 128
N_GCH = N_SHARD // GCHUNK  # 256
# growing slab schedule: the gram is PE-paced at ~114ns/chunk from its first
# matmul, so a small first slab starts the PE ~2.5us earlier; later slabs grow
# so DMA completion (~107ns/chunk) stays ahead of PE consumption
SLABS = [8, 8, 8, 12, 12, 16, 16, 20, 20, 24, 24, 28, 28, 32]
assert sum(SLABS) == N_GCH
OCHUNK = 512  # phase-2 matmul free dim
OSTORE = 1024  # output store width (2 KB/partition line in fp16)
LDCHUNK = 2048  # fp16 input DMA slice
N_LDCH = N_SHARD // LDCHUNK  # 16


def build_bass():
    nc = bacc.Bacc(
        "TRN2",
        target_bir_lowering=False,
        debug=False,
        num_devices=NCORES,
    )

    x_t = nc.dram_tensor("x", [P, N_SHARD], F16, kind="ExternalInput")
    xh_t = nc.dram_tensor("xh", [P, N_GCH, GCHUNK + 1], F16, kind="ExternalInput")
    wqa_t = nc.dram_tensor("wqa", [65, 64], F32, kind="ExternalInput")  # [wq|bq]^T/8
    wka_t = nc.dram_tensor("wka", [65, 64], F32, kind="ExternalInput")  # [wk|bk]^T
    wv_t = nc.dram_tensor("wv", [64, 64], F32, kind="ExternalInput")
    bv_t = nc.dram_tensor("bv", [64, 1], F32, kind="ExternalInput")
    id_t = nc.dram_tensor("ident", [128, 128], F32, kind="ExternalInput")
    out_t = nc.dram_tensor("out", [P, N_SHARD], F16, kind="ExternalOutput")

    with tile.TileContext(nc, num_cores=NCORES) as tc:
        with (
            tc.tile_pool(name="xbuf", bufs=1) as xpool,
            tc.tile_pool(name="consts", bufs=1) as cpool,
            tc.tile_pool(name="slab", bufs=6) as spool,
            tc.tile_pool(name="osb", bufs=6) as opool,
            tc.tile_pool(name="dram", bufs=2, space="DRAM") as dram,
        ):
            # ---- first gram slab before anything else ----
            slab_tiles = []
            slab0 = spool.tile([P, SLABS[0], GCHUNK + 1], F16, tag="slab0", bufs=1)
            nc.sync.dma_start(slab0[:], xh_t[:, 0 : SLABS[0], :])
            slab_tiles.append(slab0)
            xs = xpool.tile([P, N_SHARD], F16)

            # ---- constants to SBUF ----
            ident = cpool.tile([128, 128], F32)
            nc.scalar.dma_start(ident[:], id_t[:, :])
            wqa = cpool.tile([65, 64], F32)
            nc.scalar.dma_start(wqa[:], wqa_t[:, :])
            wka = cpool.tile([65, 64], F32)
            nc.scalar.dma_start(wka[:], wka_t[:, :])
            wv = cpool.tile([64, 64], F32)
            nc.scalar.dma_start(wv[:], wv_t[:, :])
            bv = cpool.tile([64, 1], F32)
            nc.scalar.dma_start(bv[:], bv_t[:, :])

            zeros_f = cpool.tile([128, 128], F32)
            nc.vector.memset(zeros_f[:], 0.0)
            qt_r = cpool.tile([128, 128], F16)

            # ---- phase 1: G_psum[:,0:128] += xT^T xT ; col 128 = row sums ----
            gs = cpool.tile([P, 65], F32)
            with tc.tile_pool(name="gacc", bufs=1, space="PSUM") as gpool:
                # host appends a ones channel to xh, so one accumulation chain
                # yields [G | s] together
                g_ps = gpool.tile([P, GCHUNK + 1], F32)
                base = SLABS[0]
                for t, L in enumerate(SLABS[1:], start=1):
                    slab = spool.tile(
                        [P, L, GCHUNK + 1], F16, tag=f"slab{t}", bufs=1,
                        name=f"slab_{t}",
                    )
                    nc.sync.dma_start(slab[:], xh_t[:, base : base + L, :])
                    slab_tiles.append(slab)
                    base += L
                for k in range(N_LDCH):
                    sl = slice(k * LDCHUNK, (k + 1) * LDCHUNK)
                    nc.sync.dma_start(xs[:, sl], x_t[:, sl])
                j = 0
                for slab, L in zip(slab_tiles, SLABS):
                    for q in range(L):
                        nc.tensor.matmul(
                            g_ps[:],
                            lhsT=slab[:, q, 0:GCHUNK],
                            rhs=slab[:, q, :],
                            start=(j == 0),
                            stop=(j == N_GCH - 1),
                        )
                        j += 1
                # split the small copies across vector and scalar so the
                # serial chain to the collective trigger is ~2us shorter
                nc.vector.tensor_copy(gs[:, 64:65], g_ps[:, 128:129])
                nc.vector.tensor_copy(gs[0:64, 0:64], g_ps[0:64, 0:64])
                nc.scalar.add(gs[64:128, 0:64], g_ps[64:128, 64:128], 0.0)

            # ---- local S partial: S_p = wqa^T G_aug_p wka (linear in G) ----
            mpool = tc.alloc_tile_pool(name="pmath", bufs=1, space="PSUM")
            # s^T row via PE transpose of the s column
            st_ps = mpool.tile([1, 128], F32, tag="m1")
            nc.tensor.transpose(st_ps[:], gs[:, 64:65], ident[:])
            st = cpool.tile([1, 128], F32)
            nc.vector.tensor_copy(st[:], st_ps[:])

            ga = []
            for b in range(B):
                g_aug = cpool.tile([65, 65], F32, tag=f"ga{b}", name=f"g_aug{b}")
                cs = slice(b * 64, (b + 1) * 64)
                nc.vector.tensor_copy(g_aug[0:64, 0:64], gs[cs, 0:64])
                nc.vector.tensor_copy(g_aug[0:64, 64:65], gs[cs, 64:65])
                nc.vector.tensor_copy(g_aug[64:65, 0:64], st[:, cs])
                nc.vector.memset(g_aug[64:65, 64:65], float(N_SHARD))
                ga.append(g_aug)

            # A_b = G_aug_b @ wka  (G_aug symmetric -> lhsT = G_aug)
            cc_in = dram.tile([P, 64], F32)
            cc_out = dram.tile([P, 64], F32, addr_space="Shared")
            s_ps = mpool.tile([P, 64], F32, tag="m2")
            s_sb = cpool.tile([P, 64], F32)
            for b in range(B):
                cs = slice(b * 64, (b + 1) * 64)
                a_ps = mpool.tile([65, 64], F32, tag="m1", name=f"a_ps{b}")
                nc.tensor.matmul(a_ps[:], lhsT=ga[b][:], rhs=wka[:])
                a_sb = cpool.tile([65, 64], F32, tag=f"asb{b}", name=f"a_sb{b}")
                if b == 0:
                    nc.vector.tensor_copy(a_sb[:], a_ps[:])
                else:
                    nc.scalar.add(a_sb[:], a_ps[:], 0.0)
                # S_b = wqa^T @ A_b   (1/8 scale folded into wqa)
                nc.tensor.matmul(s_ps[cs, :], lhsT=wqa[:], rhs=a_sb[:])
                if b == 0:
                    nc.vector.tensor_copy(s_sb[cs, :], s_ps[cs, :])
                else:
                    nc.scalar.add(s_sb[cs, :], s_ps[cs, :], 0.0)
                # half-store so the first 16KB rides the queue while the
                # second half's copy finishes
                nc.scalar.dma_start(cc_in[cs, :], s_sb[cs, :])

            # ---- AllReduce the S partials (summed in-network) ----
            nc.gpsimd.collective_compute(
                "AllReduce",
                mybir.AluOpType.add,
                replica_groups=[list(range(NCORES))],
                ins=[cc_in.opt()],
                outs=[cc_out.opt()],
            )
            sr = cpool.tile([P, 64], F32)
            nc.scalar.dma_start(sr[:], cc_out)

            # softmax rows (both batches stacked [128, 64])
            negmax = cpool.tile([P, 1], F32)
            nc.vector.reduce_max(
                negmax[:], sr[:], axis=mybir.AxisListType.X, negate=True
            )
            expv = cpool.tile([P, 64], F32)
            rowsum = cpool.tile([P, 1], F32)
            nc.scalar.activation(
                expv[:], sr[:], mybir.ActivationFunctionType.Exp,
                bias=negmax[:, 0:1], scale=1.0, accum_out=rowsum[:, 0:1],
            )
            rinv = cpool.tile([P, 1], F32)
            nc.vector.reciprocal(rinv[:], rowsum[:])
            attn = cpool.tile([P, 64], F32)
            nc.vector.tensor_scalar_mul(attn[:], expv[:], rinv[:, 0:1])

            # attn^T (one transpose: [128,64] -> [64,128] = [attn0^T | attn1^T])
            at_ps = mpool.tile([64, 128], F32, tag="m1")
            nc.tensor.transpose(at_ps[:], attn[:], ident[:])
            at_sb = cpool.tile([64, 128], F32)
            nc.vector.tensor_copy(at_sb[:], at_ps[:])

            # QT block-diag [128,128]: QT_b = wv^T attn_b^T + I
            qt_ps = mpool.tile([128, 128], F32, tag="m2")
            c_ps = mpool.tile([128, 1], F32, tag="m3")
            for b in range(B):
                cs = slice(b * 64, (b + 1) * 64)
                nc.tensor.matmul(
                    qt_ps[cs, cs], lhsT=wv[:], rhs=at_sb[:, cs],
                    start=True, stop=False,
                )
                nc.tensor.matmul(
                    qt_ps[cs, cs], lhsT=ident[0:64, 0:64], rhs=ident[0:64, 0:64],
                    start=False, stop=True,
                )
                nc.tensor.matmul(c_ps[cs, :], lhsT=at_sb[:, cs], rhs=bv[:])
            nc.vector.tensor_copy(qt_r[0:64, 64:128], zeros_f[0:64, 64:128])
            nc.vector.tensor_copy(qt_r[64:128, 0:64], zeros_f[64:128, 0:64])
            for b in range(B):
                cs = slice(b * 64, (b + 1) * 64)
                nc.vector.tensor_copy(qt_r[cs, cs], qt_ps[cs, cs])
            cvec = cpool.tile([P, 1], F32)
            nc.vector.tensor_copy(cvec[:], c_ps[:])
            mpool.release()

            # ---- phase 2: out = QT^T x + c  (fp16 matmuls, fp16 stores) ----
            with tc.tile_pool(name="ops", bufs=6, space="PSUM") as oppool:
                for k in range(N_SHARD // OSTORE):
                    osb = opool.tile([P, OSTORE], F16, tag="osb", name="osb")
                    for h in range(OSTORE // OCHUNK):
                        sl = slice(k * OSTORE + h * OCHUNK, k * OSTORE + (h + 1) * OCHUNK)
                        o_ps = oppool.tile([P, OCHUNK], F32, tag="o", name="o_ps")
                        nc.tensor.matmul(o_ps[:], lhsT=qt_r[:], rhs=xs[:, sl])
                        oslice = osb[:, h * OCHUNK : (h + 1) * OCHUNK]
                        if h % 2 == 0:
                            nc.vector.tensor_scalar_add(oslice, o_ps[:], cvec[:, 0:1])
                        else:
                            nc.scalar.add(oslice, o_ps[:], cvec[:, 0:1])
                    nc.gpsimd.dma_start(
                        out_t[:, k * OSTORE : (k + 1) * OSTORE], osb[:]
                    )

    nc.compile()
    return nc


_cached_nc = None


def kernel(x, wq, bq, wk, bk, wv, bv, _trace=False):
    global _cached_nc
    x = np.ascontiguousarray(np.asarray(x, dtype=np.float32))
    assert x.shape == (B, C, 64, 64, 64)
    xf = x.reshape(P, N_TOTAL)

    wqa = (
        np.concatenate(
            [np.asarray(wq, np.float64), np.asarray(bq, np.float64)[:, None]], axis=1
        ).T
        / 8.0
    ).astype(np.float32)  # [65, 64]
    wka = (
        np.concatenate(
            [np.asarray(wk, np.float64), np.asarray(bk, np.float64)[:, None]], axis=1
        ).T
    ).astype(np.float32)  # [65, 64]
    wv32 = np.ascontiguousarray(np.asarray(wv, np.float32))
    bv32 = np.ascontiguousarray(np.asarray(bv, np.float32).reshape(64, 1))
    ident = np.eye(128, dtype=np.float32)

    in_maps = []
    for i in range(NCORES):
        sl = slice(i * N_SHARD, (i + 1) * N_SHARD)
        xsh = np.ascontiguousarray(xf[:, sl].astype(np.float16))
        # xh[p, q, c] = x[c, q*128 + p] in fp16 (gram operand, n on partitions)
        xh = xsh.astype(np.float16).reshape(P, N_GCH, GCHUNK).transpose(2, 1, 0)
        xh = np.ascontiguousarray(
            np.concatenate(
                [xh, np.ones((GCHUNK, N_GCH, 1), np.float16)], axis=2
            )
        )
        in_maps.append(
            {
                "x": xsh,
                "xh": xh,
                "wqa": wqa,
                "wka": wka,
                "wv": wv32,
                "bv": bv32,
                "ident": ident,
            }
        )

    if _cached_nc is None:
        _cached_nc = build_bass()
    nc = _cached_nc

    res = bass_utils.run_bass_kernel_spmd(
        nc, in_maps, core_ids=list(range(NCORES)), trace=_trace
    )
    kernel._last_results = res

    out = np.empty((P, N_TOTAL), dtype=np.float32)
    for i in range(NCORES):
        out[:, i * N_SHARD : (i + 1) * N_SHARD] = res.results[i]["out"].astype(
            np.float32
        )
    return out.reshape(B, C, 64, 64, 64)


kernel._last_results = None
